# revision 2
# baseline (speedup 1.0000x reference)
"""Trainium2 Bass kernel for nn_AlignerOT — v3 (no hot-loop tile churn).

Same math as v1: per-sample log-domain Sinkhorn via kernel-space
iterations on M = exp(pre + BR), restabilized every 10 iters.

v3 structure (changes vs v1/v2):
  - ZERO tile-pool allocations inside hot loops: pool acquire/release was
    measured at ~2.15us per call on HW (sync-engine round trip); all psum
    and sbuf working tiles are allocated once and reused.
  - PSUM map (8 banks): pss0/pss1 (stream rows; h0 at partition sbase via
    quadrant sbase, h1 at partition sbase+64 via quadrant sbase+64 — one
    bank per sample-pair), psc0/psc1 (trick matmuls + row building),
    reb0..reb3 (rebuild tiles, one per sample slot).
  - GROUP=4 samples in flight: stream quadrant pair = 32*(gslot%2) and
    +64; pack-row strip = 32*gslot (4-way row-tile concurrency in the
    rebuild); psum/psc shared by (gslot%2) with temporal WAR chaining.
  - K=3+K=2 packed rebuild (rowterm folded into the K=2 ones-pack rows),
    direct MT build via the transposed pack with per-partition bias,
    accum_out row sums => first u-side of each segment is free,
    h-split tail copies (ACT h0 / DVE h1).
Sharding: data-parallel over N (16/core); AllReduce sum of P*D; out rows
per-core; host concat.
"""

import numpy as np

N_CORES = 8
N_GLOB = 128
NS = N_GLOB // N_CORES   # 16
S_IN = 768
SC = S_IN // 128         # 6
D = 1024
JC = D // 128            # 8
EPS = 0.1
SCALE = 300.0
GAMMA = SCALE / EPS
RT2G = float(np.sqrt(2.0 * GAMMA))
RTG = float(np.sqrt(GAMMA))
N_ITERS = 50
N_SEG = 5
GROUP = 4

_cache = {}


def build(n_iters=N_ITERS, n_seg=N_SEG, ns=NS, group=GROUP, n_cores=N_CORES,
          skip_collective=False, sides_mode="full"):
    import concourse.bass as bass
    import concourse.bacc as bacc
    import concourse.tile as tile
    import concourse.mybir as mybir
    from concourse.masks import make_identity

    fp32 = mybir.dt.float32
    bf16 = mybir.dt.bfloat16
    fp16 = mybir.dt.float16
    AF = mybir.ActivationFunctionType
    ALU = mybir.AluOpType
    AX = mybir.AxisListType
    ET = mybir.EngineType

    nc = bacc.Bacc("TRN2", target_bir_lowering=False, debug=False,
                   num_devices=n_cores)

    x_d = nc.dram_tensor("x", [ns, S_IN], fp32, kind="ExternalInput")
    y_d = nc.dram_tensor("y", [ns, D], fp32, kind="ExternalInput")
    w_d = nc.dram_tensor("w", [D, S_IN], fp32, kind="ExternalInput")
    b_d = nc.dram_tensor("bvec", [1, D], fp32, kind="ExternalInput")
    delta_d = nc.dram_tensor("delta", [D, D], fp32, kind="ExternalInput")
    out_d = nc.dram_tensor("out", [ns, D], fp32, kind="ExternalOutput")

    assert ns % group == 0
    assert n_iters % n_seg == 0
    seg_len = n_iters // n_seg

    # packed per-slot column vectors inside svf [128, 64] f32:
    UCF, VCF, BRC, BCC, LNX, S2C, AUXC = (
        slice(0, 8), slice(8, 16), slice(16, 24), slice(24, 32),
        slice(32, 40), slice(40, 48), slice(48, 56))
    RM = slice(56, 58)

    with tile.TileContext(nc) as tc:
        with (
            tc.tile_pool(name="const", bufs=1) as cpool,
            tc.tile_pool(name="rdata", bufs=1) as rpool,
            tc.tile_pool(name="acc", bufs=1) as apool,
            tc.tile_pool(name="dram", bufs=2, space="DRAM") as dpool,
        ):
            identh = cpool.tile([128, 128], fp16)
            make_identity(nc, identh[:])
            oneb = cpool.tile([1, 1], bf16)
            nc.gpsimd.memset(oneb[:], 1.0)
            ones2 = cpool.tile([128, 128], fp16)
            nc.gpsimd.memset(ones2[:], 1.0)

            # ---------------- phase 1: src = X @ W.T + b --------------------
            src_sb = rpool.tile([ns, D], fp32)
            y_sb = rpool.tile([ns, D], fp32)
            nc.sync.dma_start(y_sb[:], y_d.ap()[:])
            srccol = rpool.tile([128, ns, JC], fp32)
            with (
                tc.tile_pool(name="wls", bufs=1) as wpool,
                tc.tile_pool(name="psp1", bufs=2, space="PSUM") as psp1,
            ):
                identf = wpool.tile([128, 128], fp32)
                make_identity(nc, identf[:])
                xt = wpool.tile([128, SC, ns], fp32)
                for sc in range(SC):
                    nc.sync.dma_start(
                        xt[:, sc, :],
                        x_d.ap()[:, sc * 128:(sc + 1) * 128].rearrange(
                            "n p -> p n"))
                ones16 = wpool.tile([1, ns], fp32)
                nc.gpsimd.memset(ones16[:], 1.0)
                b_row = wpool.tile([1, D], fp32)
                nc.sync.dma_start(b_row[:], b_d.ap()[:])
                w_sb = wpool.tile([128, JC, S_IN], fp32)
                nc.sync.dma_start(
                    w_sb[:], w_d.ap().rearrange("(dc p) s -> p dc s", p=128))
                wt = wpool.tile([128, SC, D], fp32)
                for dc in range(JC):
                    for sc in range(SC):
                        pst = psp1.tile([128, 512], fp32, tag="p1")
                        nc.tensor.transpose(
                            pst[:, :128],
                            w_sb[:, dc, sc * 128:(sc + 1) * 128], identf[:])
                        nc.vector.tensor_copy(
                            wt[:, sc, dc * 128:(dc + 1) * 128], pst[:, :128])
                for h in range(2):
                    ps_src = psp1.tile([128, 512], fp32, tag="p1")
                    for sc in range(SC):
                        nc.tensor.matmul(
                            ps_src[:ns], xt[:, sc, :],
                            wt[:, sc, h * 512:(h + 1) * 512],
                            start=(sc == 0), stop=False)
                    nc.tensor.matmul(
                        ps_src[:ns], ones16[:],
                        b_row[:, h * 512:(h + 1) * 512],
                        start=False, stop=True)
                    nc.scalar.activation(
                        src_sb[:, h * 512:(h + 1) * 512], ps_src[:ns],
                        AF.Copy)
                for c in range(JC):
                    pst = psp1.tile([128, 512], fp32, tag="p1")
                    nc.tensor.transpose(
                        pst[:, :ns], src_sb[:, c * 128:(c + 1) * 128],
                        identf[:ns, :ns])
                    nc.vector.tensor_copy(srccol[:, :, c], pst[:, :ns])

            pacc = apool.tile([128, JC, D], fp32)
            nc.gpsimd.memset(pacc[:], 0.0)

            # ------------- phase 2: per-sample Sinkhorn ---------------------
            with (
                tc.tile_pool(name="mats", bufs=1) as mpool,
                tc.tile_pool(name="rows", bufs=1) as wrow,
                tc.tile_pool(name="vecs", bufs=1) as vpool,
                tc.tile_pool(name="psfx", bufs=1, space="PSUM") as psfx,
            ):
                # ---- fixed PSUM map (8 banks) ----
                # pss[parity][h]: stream rows; psc[parity]: tricks/rows;
                # reb[parity]: rebuild tiles
                pss_t = []
                for pn in range(4):
                    pt_ = psfx.tile([128, 512], fp32, name=f"pss{pn}")
                    pss_t.append(pt_)
                psc0 = psfx.tile([128, 512], fp32)
                psc1 = psfx.tile([128, 512], fp32)
                reb0 = psfx.tile([128, 512], fp32)
                reb1 = psfx.tile([128, 512], fp32)
                pss_by_par = [[pss_t[0], pss_t[1]], [pss_t[2], pss_t[3]]]
                psc_by_par = [psc0, psc1]
                reb = [reb0, reb1, reb0, reb1]

                # ---- fixed SBUF pack banks / per-slot state ----
                ybank = wrow.tile([128, D], fp16)
                sbank = wrow.tile([128, D], fp16)
                tbank = wrow.tile([128, D], fp16)
                ubank = wrow.tile([128, D], fp16)
                rbank = wrow.tile([128, D], fp16)
                brbank = wrow.tile([128, D], fp16)
                scr = wrow.tile([1, D], fp32)
                sc2 = wrow.tile([1, D], fp32)
                s16 = wrow.tile([1, D], fp16)
                ptiles = []
                for g in range(2):
                    pt = vpool.tile([128, 512], fp32, name=f"pt{g}")
                    ptiles.append(pt)
                slot = []
                for g in range(group):
                    M = mpool.tile([128, JC, D], bf16, name=f"M{g}")
                    MT = mpool.tile([128, JC, D], bf16, name=f"MT{g}")
                    svf = vpool.tile([128, 64], fp32, name=f"svf{g}")
                    svb = vpool.tile([128, 16], bf16, name=f"svb{g}")
                    svh = vpool.tile([128, 8, 2], fp16, name=f"svh{g}")
                    srow = vpool.tile([1, D], bf16, name=f"srow{g}")
                    rs = vpool.tile([128, 16], fp32, name=f"rs{g}")
                    slot.append(dict(
                        M=M, MT=MT, svf=svf, svb=svb, svh=svh, srow=srow,
                        rs=rs, ptile=ptiles[g % 2], base=32 * g,
                        sbase=32 * (g % 2)))

                def col_to_rows(d, cols_f32, bank):
                    """fp16 hi/lo split of [128,8] f32 cols into bank rows
                    base+0/+1 via trick MMs at aligned psum partitions."""
                    base = d["base"]
                    svh = d["svh"]
                    nc.vector.tensor_copy(svh[:, :, 0], cols_f32)
                    nc.vector.tensor_copy(d["svf"][:, LNX], svh[:, :, 0])
                    nc.vector.tensor_sub(d["svf"][:, LNX], cols_f32,
                                         d["svf"][:, LNX])
                    nc.vector.tensor_copy(svh[:, :, 1], d["svf"][:, LNX])
                    for h in range(2):
                        psr = psc_by_par[h]
                        for c in range(4):
                            cc = h * 4 + c
                            nc.tensor.matmul(
                                psr[base:base + 2, c * 128:(c + 1) * 128],
                                svh[:, cc, :], identh[:],
                                start=True, stop=True,
                                tile_position=(0, base))
                        nc.vector.tensor_copy(
                            bank[base:base + 2, h * 512:(h + 1) * 512],
                            psr[base:base + 2, :])

                def pre_mm(d, jc, h, ps):
                    base = d["base"]
                    ja, jb = jc * 128, (jc + 1) * 128
                    ha, hb = h * 512, (h + 1) * 512
                    nc.tensor.matmul(ps[:], ybank[base:base + 3, ja:jb],
                                     sbank[base:base + 3, ha:hb],
                                     start=True, stop=False,
                                     tile_position=(base, 0))
                    nc.tensor.matmul(ps[:], ones2[base:base + 2, 0:128],
                                     rbank[base:base + 2, ha:hb],
                                     start=False, stop=True,
                                     tile_position=(base, 0))

                def pret_mm(d, kb, h, ps):
                    base = d["base"]
                    ka, kb_ = kb * 128, (kb + 1) * 128
                    ha, hb = h * 512, (h + 1) * 512
                    nc.tensor.matmul(ps[:], tbank[base:base + 3, ka:kb_],
                                     ubank[base:base + 3, ha:hb],
                                     start=True, stop=False,
                                     tile_position=(base, 0))
                    nc.tensor.matmul(ps[:], ones2[base:base + 2, 0:128],
                                     brbank[base:base + 2, ha:hb],
                                     start=False, stop=True,
                                     tile_position=(base, 0))

                def side_pair(gidx, use_mt, ucol_sel):
                    """one matvec side for the group.  Samples (0,1) and
                    (2,3) share psum tiles by parity, so the group is
                    processed pair-by-pair: pair-1 streams are emitted
                    after pair-0 copies (WAR chains the bank reuse)."""
                    for p0 in range(0, group, 2):
                        ii = gidx[p0:p0 + 2]
                        for i in ii:
                            d = slot[i % group]
                            sb = d["sbase"]
                            mat = d["MT"] if use_mt else d["M"]
                            invec = (d["svb"][:, 8:16] if ucol_sel == "v"
                                     else d["svb"][:, 0:8])
                            pssh = pss_by_par[(i % group) % 2]
                            for h in range(2):
                                par = sb + 64 * h
                                for kc in range(JC):
                                    nc.tensor.matmul(
                                        pssh[h][par:par + 1, :],
                                        invec[:, kc:kc + 1],
                                        mat[:, kc, h * 512:(h + 1) * 512],
                                        start=(kc == 0),
                                        stop=(kc == JC - 1),
                                        tile_position=(0, par))
                        if sides_mode == "streams":
                            continue
                        for i in ii:
                            d = slot[i % group]
                            sb = d["sbase"]
                            pssh = pss_by_par[(i % group) % 2]
                            nc.scalar.activation(
                                d["srow"][:, 0:512],
                                pssh[0][sb:sb + 1, :], AF.Copy)
                            nc.vector.tensor_copy(
                                d["srow"][:, 512:1024],
                                pssh[1][sb + 64:sb + 65, :])
                    if sides_mode in ("streams", "nocols"):
                        return
                    for i in gidx:
                        d = slot[i % group]
                        psc = psc_by_par[(i % group) % 2]
                        for c in range(JC):
                            nc.tensor.matmul(
                                psc[:, c:c + 1],
                                d["srow"][0:1, c * 128:(c + 1) * 128],
                                oneb[:], start=True, stop=True)
                        out_cols = d["svf"][:, UCF if ucol_sel == "v"
                                            else VCF]
                        out_colsb = (d["svb"][:, 0:8] if ucol_sel == "v"
                                     else d["svb"][:, 8:16])
                        nc.vector.reciprocal(out_cols[:], psc[:, :JC])
                        nc.vector.tensor_copy(out_colsb[:], out_cols[:])

                for g0 in range(0, ns, group):
                    gidx = list(range(g0, g0 + group))
                    for i in gidx:
                        d = slot[i % group]
                        base = d["base"]
                        svf, svb = d["svf"], d["svb"]
                        # fp16 hi/lo splits of sqrt(2g)*Y and sqrt(2g)*src
                        for (srcrow, hi_dsts, lo_dsts) in (
                                (y_sb,
                                 ((ybank, 0), (ybank, 1), (ubank, 0),
                                  (ubank, 2)),
                                 ((ybank, 2), (ubank, 1))),
                                (src_sb,
                                 ((sbank, 0), (sbank, 2), (tbank, 0),
                                  (tbank, 1)),
                                 ((sbank, 1), (tbank, 2)))):
                            nc.sync.dma_start(scr[:], srcrow[i:i + 1, :])
                            nc.vector.tensor_scalar_mul(scr[:], scr[:],
                                                        RT2G)
                            nc.vector.tensor_copy(s16[:], scr[:])   # hi
                            for (bank, r) in hi_dsts:
                                nc.sync.dma_start(
                                    bank[base + r:base + r + 1, :], s16[:])
                            nc.vector.tensor_copy(sc2[:], s16[:])
                            nc.vector.tensor_sub(scr[:], scr[:], sc2[:])
                            nc.vector.tensor_copy(s16[:], scr[:])   # lo
                            for (bank, r) in lo_dsts:
                                nc.sync.dma_start(
                                    bank[base + r:base + r + 1, :], s16[:])
                        # seg-0 nbc rows = -gamma*src^2 (BC=0), hi/lo
                        nc.sync.dma_start(scr[:], src_sb[i:i + 1, :])
                        nc.scalar.activation(sc2[:], scr[:], AF.Square,
                                             scale=RTG)
                        nc.vector.tensor_scalar_mul(scr[:], sc2[:], -1.0)
                        nc.vector.tensor_copy(s16[:], scr[:])
                        nc.sync.dma_start(
                            rbank[base + 0:base + 1, :], s16[:])
                        nc.vector.tensor_copy(sc2[:], s16[:])
                        nc.vector.tensor_sub(scr[:], scr[:], sc2[:])
                        nc.vector.tensor_copy(s16[:], scr[:])
                        nc.sync.dma_start(
                            rbank[base + 1:base + 2, :], s16[:])
                        # s2 col = gamma*src^2
                        nc.scalar.activation(svf[:, S2C], srccol[:, i, :],
                                             AF.Square, scale=RTG)
                        nc.gpsimd.memset(svf[:, BCC], 0.0)
                        if sides_mode != "full":
                            nc.gpsimd.memset(svf[:, UCF], 1.0)
                            nc.gpsimd.memset(svf[:, VCF], 1.0)
                            nc.vector.tensor_copy(svb[:, 0:8],
                                                  svf[:, UCF])
                            nc.vector.tensor_copy(svb[:, 8:16],
                                                  svf[:, VCF])

                    # ---- init pass: BRC = -max_k(pre with BC=0) ----
                    for jc in range(JC):
                        for i in gidx:
                            g = i % group
                            d = slot[g]
                            svf = d["svf"]
                            for h in range(2):
                                ps = reb[g]
                                pre_mm(d, jc, h, ps)
                                nc.vector.tensor_reduce(
                                    out=svf[:, RM][:, h:h + 1], in_=ps[:],
                                    op=ALU.max, axis=AX.X)
                            nc.vector.tensor_max(
                                svf[:, RM][:, 0:1], svf[:, RM][:, 0:1],
                                svf[:, RM][:, 1:2])
                            nc.vector.tensor_scalar_mul(
                                svf[:, BRC][:, jc:jc + 1],
                                svf[:, RM][:, 0:1], -1.0)

                    def seg_body():
                        for i in gidx:
                            d = slot[i % group]
                            svf = d["svf"]
                            nc.vector.tensor_sub(svf[:, AUXC], svf[:, BCC],
                                                 svf[:, S2C])
                            col_to_rows(d, svf[:, AUXC], rbank)
                            col_to_rows(d, svf[:, BRC], brbank)
                        # M = exp(pre + BR), accum_out row sums
                        for jc in range(JC):
                            for h in range(2):
                                for i in gidx:
                                    g = i % group
                                    d = slot[g]
                                    svf = d["svf"]
                                    ps = reb[g]
                                    pre_mm(d, jc, h, ps)
                                    nc.scalar.activation(
                                        d["M"][:, jc, h * 512:(h + 1) * 512],
                                        ps[:], AF.Exp,
                                        bias=svf[:, BRC][:, jc:jc + 1],
                                        accum_out=d["rs"][:, 8 * h + jc:
                                                          8 * h + jc + 1])
                        # MT = exp(pre^T + BR), bias = (BC - s2)[k]
                        for kb in range(JC):
                            for h in range(2):
                                for i in gidx:
                                    g = i % group
                                    d = slot[g]
                                    svf = d["svf"]
                                    ps = reb[g]
                                    pret_mm(d, kb, h, ps)
                                    nc.scalar.activation(
                                        d["MT"][:, kb,
                                                h * 512:(h + 1) * 512],
                                        ps[:], AF.Exp,
                                        bias=svf[:, AUXC][:, kb:kb + 1])
                        for i in gidx:
                            d = slot[i % group]
                            svf, svb, rs = d["svf"], d["svb"], d["rs"]
                            # free first u-side: u1 = 1/rowsum(M)
                            nc.vector.tensor_add(svf[:, LNX], rs[:, 0:8],
                                                 rs[:, 8:16])
                            nc.vector.reciprocal(svf[:, UCF], svf[:, LNX])
                            nc.vector.tensor_copy(svb[:, 0:8],
                                                  svf[:, UCF])
                        for t in range(seg_len):
                            if t > 0:
                                side_pair(gidx, use_mt=True, ucol_sel="v")
                            side_pair(gidx, use_mt=False, ucol_sel="u")
                        for i in gidx:
                            svf = slot[i % group]["svf"]
                            nc.scalar.activation(svf[:, LNX], svf[:, UCF],
                                                 AF.Ln)
                            nc.vector.tensor_add(svf[:, BRC], svf[:, BRC],
                                                 svf[:, LNX])
                            nc.scalar.activation(svf[:, LNX], svf[:, VCF],
                                                 AF.Ln)
                            nc.vector.tensor_add(svf[:, BCC], svf[:, BCC],
                                                 svf[:, LNX])

                    if n_seg > 1:
                        with tc.For_i(0, n_seg, 1, hint_engines=(ET.PE,)):
                            seg_body()
                    else:
                        seg_body()

                    # ---- P accumulation: D*P = exp(pre + BRC) --------------
                    for i in gidx:
                        d = slot[i % group]
                        svf = d["svf"]
                        nc.vector.tensor_sub(svf[:, AUXC], svf[:, BCC],
                                             svf[:, S2C])
                        col_to_rows(d, svf[:, AUXC], rbank)
                    for jc in range(JC):
                        for h in range(2):
                            for i in gidx:
                                g = i % group
                                d = slot[g]
                                svf = d["svf"]
                                ps = reb[g]
                                pre_mm(d, jc, h, ps)
                                nc.scalar.activation(
                                    d["ptile"][:], ps[:], AF.Exp,
                                    bias=svf[:, BRC][:, jc:jc + 1])
                                nc.vector.tensor_add(
                                    pacc[:, jc, h * 512:(h + 1) * 512],
                                    pacc[:, jc, h * 512:(h + 1) * 512],
                                    d["ptile"][:])

            # ------------- phase 3: AllReduce + finale ----------------------
            pacc_b = dpool.tile([D, D], fp32)
            pall_b = dpool.tile(
                [D, D], fp32,
                addr_space="Shared" if n_cores > 4 else "Local")
            nc.sync.dma_start(
                pacc_b[:].rearrange("(jc p) k -> p jc k", p=128), pacc[:])
            if skip_collective:
                nc.sync.dma_start(pall_b[:], pacc_b[:])
            else:
                nc.gpsimd.collective_compute(
                    "AllReduce", ALU.add,
                    replica_groups=[list(range(n_cores))],
                    ins=[pacc_b.opt()], outs=[pall_b.opt()],
                )
            with (
                tc.tile_pool(name="fin", bufs=1) as fpool,
                tc.tile_pool(name="psp3", bufs=2, space="PSUM") as psp3,
            ):
                ot = fpool.tile([128, JC, D], fp32)
                nc.sync.dma_start(
                    ot[:], pall_b[:].rearrange("(jc p) k -> p jc k", p=128))
                dl = fpool.tile([128, JC, D], fp32)
                nc.sync.dma_start(
                    dl[:],
                    delta_d.ap().rearrange("(jc p) k -> p jc k", p=128))
                nc.vector.tensor_scalar_mul(ot[:], ot[:], SCALE / N_GLOB)
                nc.vector.tensor_add(ot[:], ot[:], dl[:])
                out_sb = fpool.tile([ns, D], fp32)
                for h in range(2):
                    pso = psp3.tile([128, 512], fp32, tag="p3")
                    for jc in range(JC):
                        nc.tensor.matmul(
                            pso[:ns], srccol[:, :, jc],
                            ot[:, jc, h * 512:(h + 1) * 512],
                            start=(jc == 0), stop=(jc == JC - 1))
                    nc.scalar.activation(
                        out_sb[:, h * 512:(h + 1) * 512], pso[:ns], AF.Copy)
                nc.sync.dma_start(out_d.ap()[:], out_sb[:])

    nc.compile()
    return nc


def kernel(**inputs):
    X = np.ascontiguousarray(inputs["X"], np.float32)
    Y = np.ascontiguousarray(inputs["Y"], np.float32)
    W = np.ascontiguousarray(inputs["W"], np.float32)
    b = np.ascontiguousarray(inputs["b"], np.float32).reshape(1, D)
    delta = np.ascontiguousarray(inputs["delta_ot"], np.float32)

    from concourse import bass_utils

    if "nc" not in _cache:
        _cache["nc"] = build()
    nc = _cache["nc"]

    in_maps = []
    for c in range(N_CORES):
        sl = slice(c * NS, (c + 1) * NS)
        in_maps.append({
            "x": X[sl], "y": Y[sl], "w": W, "bvec": b, "delta": delta,
        })
    res = bass_utils.run_bass_kernel_spmd(
        nc, in_maps, core_ids=list(range(N_CORES)))
    out = np.concatenate([res.results[c]["out"] for c in range(N_CORES)],
                         axis=0)
    return out.astype(np.float32)


if __name__ == "__main__":
    import reference
    ins = reference.setup_inputs()
    ins = {k: np.asarray(v) for k, v in ins.items()}
    got = kernel(**ins)
    print("out", got.shape, got.dtype)
